# revision 21
# baseline (speedup 1.0000x reference)
"""Trainium2 Bass kernel for nn_CustomRetrieverModel (retrieval_knn).

Late-interaction retriever scoring:
  sim4d = l2n(q_tok) @ l2n(d_tok * punct).T  -> max over doc tokens
  -> valid-weighted mean over query tokens -> avg_sim (B, M)
  logits = shuffle(avg_sim) * shuffle(Wq) * exp(log_inv_t)
  with Wq from L2-normalized CLS vectors: (center - min cand)/2.

Sharding: data-parallel over the M (document) axis. Each of the 8 cores
scores all B=32 queries against M/8 = 8 docs; q-side inputs replicated,
host concatenates the per-core (B, 8) logits and applies the even/odd
column shuffle (a pure output permutation commutes with the elementwise
finale).

Device-side plan (per core), v4:
  - Matmul operands arrive HOST-PRE-TRANSPOSED in bf16 as
    (128, 4, 6, 512) = (h%128, col-group, h-chunk, col): the PE does no
    layout transposes and streams 1 row/cycle at 2.4 GHz.
  - The two *hardware* DGE rings (SP + Activation) run ~105-140 GB/s
    each and gate the start, so the first dT/qT group is split in half
    across both rings and every transfer is kick-ordered by its compute
    deadline; latency-tolerant bulk (last dT group, fp8 naturals for q
    norms, CLS, n_valid ids) rides the software DGE queues.
  - Row norms come from fp8-e4m3 natural-layout copies via ACT square +
    free-axis accumulate (norm error ~0.2%, well inside tolerance).
  - 1/||x|| uses the ACT Rsqrt table + one DVE Newton step.  Square,
    Copy and Rsqrt share one ACT table set (reciprocal_sqrt_and_small),
    so after an early warm-up there are no table reloads; the lone Exp
    runs first, while DMA is still streaming.
  - q is not normalized on device: max over doc tokens commutes with the
    positive row scale 1/||q||, folded into the weighted-sum weights.
  - d columns are scaled by mask/||d||: groups 1-3 in place on the DVE
    ahead of use; group 0 applies the scale to the PSUM sim block just
    before its reduce_max, so the first matmuls depend only on raw
    dT/qT arriving -- not on the normalization chain.
  - main matmul: per (q-tile, d-group) block, 6 bf16 matmuls of
    (128x128)@(128x512) accumulate in PSUM; DVE reduce_max per doc;
    weighted-sum matmul accumulates the (32, 8) sum_sim during the last
    group sweep.
  - CLS path in bf16: matmul first, separable row/column normalization
    applied to the (32, 24) result.
  - The Tile scheduler reorders per-engine streams using an optimistic
    DMA model, so everything with late-arriving inputs (q squares,
    build_W, CLS, finale) carries a tile_wait_until() floor to keep it
    from head-of-line blocking critical FIFO positions.
  - pad d tokens are zeroed (not -1e-9-masked): only changes the max
    when every real token sims below -1e-9, an O(1e-9) absolute effect.
"""

import sys

for _p in ("/opt/trn_rl_repo",):
    if _p not in sys.path:
        sys.path.append(_p)

import contextlib
import math

import numpy as np
import ml_dtypes

import concourse.bass as bass
import concourse.tile as tile
from concourse import bacc, mybir
import concourse.bass_utils as bass_utils

# ---- problem shape (hardcoded per spec) ----
B, LQ, M, LD, H, L = 32, 64, 64, 256, 768, 3
NCORES = 8
MLOC = M // NCORES          # 8 docs per core
BQ = B * LQ                 # 2048 query rows
DR = MLOC * LD              # 2048 doc-token rows per core
KC = H // 128               # 6 contraction chunks
QT = BQ // 128              # 16 q row tiles
DT = DR // 128              # 16 d row tiles
NG = 4                      # 512-wide column groups

EPS_DIV = 1e-10
LN2 = math.log(2.0)

F32 = mybir.dt.float32
BF16 = mybir.dt.bfloat16
FP8 = mybir.dt.float8e4
I32 = mybir.dt.int32


def _emit(nc, tc, io):
    q_t = io["q_t"].ap()          # (128, 4, 6, 512) bf16  q^T, replicated
    d_t = io["d_t"].ap()          # (128, 4, 6, 512) bf16  d^T shard
    q_n = io["q_n"].ap()          # (128, 16, 768) bf16 natural q (norms)
    d_n = io["d_n"].ap()          # (128, 16, 768) bf16 natural d (norms)
    qids = io["qids"].ap()        # (32, 64)   i32    replicated
    qcls = io["qcls"].ap()        # (32, 768)  bf16   q_cls[-1] natural
    qclsT = io["qclsT"].ap()      # (128, 6, 32) bf16 q_cls[-1]^T
    dcls = io["dcls"].ap()        # (24, 768)  bf16   d_cls shard natural
    dclsT = io["dclsT"].ap()      # (128, 6, 24) bf16 d_cls shard^T
    logt = io["logt"].ap()        # (32, 1)    f32    log_inv_t replicated
    out = io["out"].ap()          # (32, 8)    f32

    AF = mybir.ActivationFunctionType
    ALU = mybir.AluOpType

    ctx = contextlib.ExitStack()
    singles = ctx.enter_context(tc.tile_pool(name="singles", bufs=1))
    smalls = ctx.enter_context(tc.tile_pool(name="smalls", bufs=4))

    # ---------- constants (gpsimd only; no DMA) ----------
    identf = singles.tile([128, 128], F32)
    nc.gpsimd.memset(identf, 1.0)
    nc.gpsimd.affine_select(
        out=identf, in_=identf, pattern=[[-1, 128]], base=0,
        channel_multiplier=1, compare_op=ALU.is_equal, fill=0.0,
    )
    onesb = singles.tile([1, 128], BF16)
    nc.gpsimd.memset(onesb, 1.0)
    ones32 = singles.tile([1, 32], F32)
    nc.gpsimd.memset(ones32, 1.0)
    warm1 = singles.tile([1, 1], F32)
    nc.gpsimd.memset(warm1, 1.0)
    bln2 = singles.tile([32, 1], F32)
    nc.gpsimd.memset(bln2, -LN2)
    W = singles.tile([128, QT, 2 * QT], F32)  # block-diagonal (128, 16, 32)
    nc.gpsimd.memset(W, 0.0)

    # ---------- persistent SBUF ----------
    qss = singles.tile([128, QT], F32)
    dss = singles.tile([128, DT], F32)
    qT = singles.tile([128, NG, KC, 512], BF16)
    dT = singles.tile([128, NG, KC, 512], BF16)
    S = singles.tile([128, NG, 512], BF16)
    dmsk = singles.tile([128, 2 * DT], I32)   # [dids | dpunct] packed
    qid_t = singles.tile([128, QT], I32)
    qid_n = smalls.tile([32, 64], I32)
    lt_t = smalls.tile([32, 1], F32)
    dn0 = singles.tile([128, 4, H], BF16)      # natural d, group 0
    dn123 = singles.tile([128, 12, H], BF16)   # natural d, groups 1-3
    qn = singles.tile([128, QT, H], BF16)      # natural q, all tiles

    # -------- rsqrt: ACT Sqrt (table shared with Square) + DVE polish ----
    def rsqrt_nt(dst, ss, pool):
        shape = list(ss.shape)
        n0 = pool.tile(shape, F32, tag="rsq_n0")
        nc.scalar.sqrt(n0, ss)
        nc.vector.tensor_scalar_max(n0, n0, 1e-30)
        r0 = pool.tile(shape, F32, tag="rsq_r0")
        nc.vector.reciprocal(r0, n0)
        t = pool.tile(shape, F32, tag="rsq_t")
        nc.vector.tensor_mul(t, ss, r0)
        nc.vector.tensor_add(t, t, n0)
        nc.vector.tensor_scalar(t, t, 0.5, 1e-12, op0=ALU.mult, op1=ALU.max)
        nc.vector.reciprocal(dst, t)

    with tc.tile_pool(name="sqscr", bufs=2) as sqscr, \
         tc.tile_pool(name="clsp", bufs=1) as clsp, \
         tc.tile_pool(name="mm", bufs=5, space="PSUM") as mm_pool, \
         tc.tile_pool(name="aux", bufs=2, space="PSUM") as aux_ps, \
         tc.tile_pool(name="ws", bufs=1, space="PSUM") as ws_pool, \
         tc.tile_pool(name="maxs", bufs=QT) as maxs_pool:

        maxs_tiles = [None] * QT

        def d_squares(src, cols, g):
            for j in range(4):
                c = 4 * g + j
                scr = sqscr.tile([128, H], BF16, tag="sq")
                nc.scalar.activation(scr, src[:, cols + j, :], AF.Square,
                                     accum_out=dss[:, c:c + 1])

        def q_squares(gg):
            for j in range(4):
                c = 4 * gg + j
                scr = sqscr.tile([128, H], BF16, tag="sq")
                nc.scalar.activation(scr, qn[:, c, :], AF.Square,
                                     accum_out=qss[:, c:c + 1])

        def d_scale(g):
            # s = mask / ||d|| for this group's 4 tiles
            sl = slice(4 * g, 4 * g + 4)
            s4 = smalls.tile([128, 4], F32, tag="s4")
            rsqrt_nt(s4, dss[:, sl], smalls)
            nc.vector.tensor_mul(s4, s4, dmask_f[:, sl])
            # relayout (128, 4) -> scale row (1, 512) -> broadcast (128, 512)
            st = smalls.tile([1, 512], BF16, tag="st")
            for c in range(4):
                stp = aux_ps.tile([1, 128], F32, tag="aux")
                nc.tensor.transpose(stp, s4[:, c:c + 1], identf)
                nc.vector.tensor_copy(st[0:1, c * 128:(c + 1) * 128], stp)
            bc = aux_ps.tile([128, 512], F32, tag="aux")
            nc.tensor.matmul(bc, onesb, st, start=True, stop=True)
            nc.scalar.copy(S[:, g], bc)
            if g > 0:
                # in-place column scale of this group's dT chunks
                for k in range(KC):
                    nc.vector.tensor_mul(dT[:, g, k], dT[:, g, k], S[:, g])

        def main_block(qc, g):
            if maxs_tiles[qc] is None:
                maxs_tiles[qc] = maxs_pool.tile([128, MLOC], F32, tag="maxs",
                                                name=f"maxs{qc}")
            qg, sub = divmod(qc, 4)
            ps = mm_pool.tile([128, 512], F32, tag="mm")
            for k in range(KC):
                nc.tensor.matmul(ps, qT[:, qg, k, sub * 128:(sub + 1) * 128],
                                 dT[:, g, k],
                                 start=(k == 0), stop=(k == KC - 1))
            if g == 0:
                # group 0 runs on raw dT; apply mask/||d|| on the sim block
                nc.vector.tensor_mul(ps, ps, S[:, 0])
            nc.vector.reduce_max(
                maxs_tiles[qc][:, 2 * g:2 * g + 2],
                ps[:].rearrange("p (d l) -> p d l", l=LD),
                axis=mybir.AxisListType.X)

        def build_W():
            # q_valid mask and weighted-sum weights q_valid/||q||
            qv = smalls.tile([128, QT], F32, name="qv")
            nc.vector.tensor_scalar(qv, qid_t, 0.0, None, op0=ALU.is_equal)
            nc.vector.tensor_scalar(qv, qv, -1.0, 1.0,
                                    op0=ALU.mult, op1=ALU.add)
            rq = smalls.tile([128, QT], F32, name="rq")
            rsqrt_nt(rq, qss, smalls)
            wqw = smalls.tile([128, QT], F32, name="wqw")
            nc.vector.tensor_mul(wqw, qv, rq)
            for c in range(QT):
                nc.gpsimd.tensor_copy(W[0:64, c, 2 * c:2 * c + 1],
                                      wqw[0:64, c:c + 1])
                nc.gpsimd.tensor_copy(W[64:128, c, 2 * c + 1:2 * c + 2],
                                      wqw[64:128, c:c + 1])

        def cls_block():
            qcn = clsp.tile([32, H], BF16, tag="qcn")
            nc.scalar.dma_start(qcn, qcls)
            qcT = clsp.tile([128, KC, 32], BF16, tag="qcT")
            nc.scalar.dma_start(qcT, qclsT)
            dcn = clsp.tile([24, H], BF16, tag="dcn")
            nc.sync.dma_start(dcn, dcls)
            dcT = clsp.tile([128, KC, 24], BF16, tag="dcT")
            nc.sync.dma_start(dcT, dclsT)

            qcss = smalls.tile([32, 1], F32, tag="qcss")
            scr1 = clsp.tile([32, H], BF16, tag="clsscr")
            nc.scalar.activation(scr1, qcn, AF.Square, accum_out=qcss)
            dcss = smalls.tile([24, 1], F32, tag="dcss")
            scr2 = clsp.tile([24, H], BF16, tag="clsscr24")
            nc.scalar.activation(scr2, dcn, AF.Square, accum_out=dcss)
            rqc = smalls.tile([32, 1], F32, tag="rqc")
            rsqrt_nt(rqc, qcss, smalls)
            rdc = smalls.tile([24, 1], F32, tag="rdc")
            rsqrt_nt(rdc, dcss, smalls)

            # raw (32, 24) = qcT.T @ dcT, normalized afterwards (separable)
            cp = aux_ps.tile([32, 24], F32, tag="aux")
            for k in range(KC):
                nc.tensor.matmul(cp, qcT[:, k], dcT[:, k],
                                 start=(k == 0), stop=(k == KC - 1))
            raw = smalls.tile([32, 24], F32, tag="raw")
            nc.scalar.copy(raw, cp)
            nc.vector.tensor_scalar(raw, raw, rqc, None, op0=ALU.mult)
            # rdc (24,1) -> row (1,24) -> broadcast (32,24)
            rtp = aux_ps.tile([1, 24], F32, tag="aux")
            nc.tensor.transpose(rtp, rdc, identf[0:24, 0:24])
            rdT = smalls.tile([1, 24], F32, tag="rdT")
            nc.vector.tensor_copy(rdT, rtp)
            bcp = aux_ps.tile([32, 24], F32, tag="aux")
            nc.tensor.matmul(bcp, ones32, rdT, start=True, stop=True)
            rdB = smalls.tile([32, 24], F32, tag="rdB")
            nc.scalar.copy(rdB, bcp)
            nc.vector.tensor_mul(raw, raw, rdB)

            mind = smalls.tile([32, 8], F32, tag="mind")
            nc.vector.tensor_tensor(mind, raw[:, 0:8], raw[:, 8:16],
                                    op=ALU.min)
            wq2 = smalls.tile([32, 8], F32, tag="wq2")  # center - min_doc
            nc.vector.tensor_sub(wq2, raw[:, 16:24], mind)
            return wq2

        ws_ps = ws_pool.tile([32, MLOC], F32)   # sum_sim accumulator

        # ---------- DMA kicks, deadline-ordered across both HW rings ----
        def w(ms):
            return tc.tile_wait_until(ms)

        # software DGE queue carries the bulk (it fans out over the most
        # DMA engines, ~224 GB/s observed); deadline-ordered FIFO.  High
        # priority puts the kicks ahead of the gpsimd memsets.
        with tc.high_priority():
            nc.gpsimd.dma_start(dT[:, 0], d_t[:, 0])
            with w(0.0010):
                nc.gpsimd.dma_start(qT[:, 1], q_t[:, 1])
            with w(0.0030):
                nc.gpsimd.dma_start(qT[:, 2], q_t[:, 2])
            with w(0.0060):
                nc.gpsimd.dma_start(dT[:, 1], d_t[:, 1])
            with w(0.0090):
                nc.gpsimd.dma_start(qT[:, 3], q_t[:, 3])
            with w(0.0120):
                nc.gpsimd.dma_start(dT[:, 2], d_t[:, 2])
            with w(0.0150):
                nc.gpsimd.dma_start(dT[:, 3], d_t[:, 3])
        # SP HW ring: small/early-critical + d naturals
        nc.sync.dma_start(lt_t, logt)
        nc.sync.dma_start(dmsk, io["dmsk"].ap())
        nc.sync.dma_start(dn0, d_n[:, 0:4, :])
        with w(0.0010):
            nc.sync.dma_start(qT[:, 0, 0:3], q_t[:, 0, 0:3])
        with w(0.0050):
            nc.sync.dma_start(dn123, d_n[:, 4:16, :])
        with w(0.0500):
            nc.sync.dma_start(qid_n, qids)
        # Activation HW ring: second qT0 half + q naturals
        nc.scalar.dma_start(qT[:, 0, 3:6], q_t[:, 0, 3:6])
        with w(0.0060):
            nc.scalar.dma_start(qn, q_n)
        with w(0.0200):
            nc.scalar.dma_start(qid_t, io["qids_t"].ap())

        # exp + Rsqrt table warm-up while DMA streams (one switch, early)
        it_half = smalls.tile([32, 1], F32, tag="ith")
        nc.scalar.activation(it_half, lt_t, AF.Exp, bias=bln2, scale=1.0)
        warm2 = smalls.tile([1, 1], F32, tag="warm2")
        nc.scalar.sqrt(warm2, warm1)

        # dmask = (d_ids != 0) * punct  -- first DVE work, data lands early
        dmask_f = singles.tile([128, DT], F32)
        nc.vector.tensor_scalar(dmask_f, dmsk[:, 0:DT], 0.0, None,
                                op0=ALU.is_equal)
        nc.vector.tensor_scalar(dmask_f, dmask_f, -1.0, 1.0,
                                op0=ALU.mult, op1=ALU.add)
        pun_f = smalls.tile([128, DT], F32, tag="punf")
        nc.vector.tensor_copy(pun_f, dmsk[:, DT:2 * DT])
        nc.vector.tensor_mul(dmask_f, dmask_f, pun_f)

        # group-0 normalization chain (overlaps the first main matmuls)
        d_squares(dn0, 0, 0)
        with w(0.004):
            d_scale(0)

        wq2 = None
        for g in range(NG):
            for qc in range(QT):
                main_block(qc, g)
                if g == 3:
                    nc.tensor.matmul(ws_ps, W[:, qc, :], maxs_tiles[qc],
                                     start=(qc == 0), stop=(qc == QT - 1))
                if g == 0:
                    if qc == 1:
                        d_squares(dn123, 0, 1)
                    elif qc == 5:
                        d_scale(1)
                    elif qc == 7:
                        with w(0.020):
                            q_squares(0)
                    elif qc == 9:
                        with w(0.024):
                            q_squares(1)
                    elif qc == 11:
                        with w(0.028):
                            q_squares(2)
                    elif qc == 13:
                        with w(0.032):
                            q_squares(3)
                elif g == 1:
                    if qc == 1:
                        d_squares(dn123, 4, 2)
                    elif qc == 3:
                        d_scale(2)
                    elif qc == 10:
                        with w(0.045):
                            wq2 = cls_block()
                elif g == 2:
                    if qc == 1:
                        d_squares(dn123, 8, 3)
                    elif qc == 3:
                        d_scale(3)
            if g == 0:
                with w(0.038):
                    build_W()

        # ---------- finale ----------
        with w(0.080):
            # n_valid: 64 - sum(q_ids == 0)
            qv_n = smalls.tile([32, 64], F32, tag="qvn")
            nc.vector.tensor_scalar(qv_n, qid_n, 0.0, None, op0=ALU.is_equal)
            nv_eq = smalls.tile([32, 1], F32, tag="nveq")
            nc.vector.reduce_sum(nv_eq, qv_n, axis=mybir.AxisListType.X)
            rnv = smalls.tile([32, 1], F32, tag="rnv")
            nc.vector.tensor_scalar(rnv, nv_eq, -1.0, 64.0 + EPS_DIV,
                                    op0=ALU.mult, op1=ALU.add)
            nc.vector.reciprocal(rnv, rnv)

        avg = smalls.tile([32, 8], F32, tag="avg")
        nc.vector.tensor_scalar(avg, ws_ps, rnv, None, op0=ALU.mult)
        nc.vector.tensor_mul(avg, avg, wq2)
        outt = smalls.tile([32, 8], F32, tag="outt")
        nc.vector.tensor_scalar(outt, avg, it_half, None, op0=ALU.mult)
        nc.sync.dma_start(out, outt)

    ctx.close()


_CACHE = {}


def _build():
    if "nc" in _CACHE:
        return _CACHE["nc"]
    nc = bacc.Bacc("TRN2", target_bir_lowering=False, debug=False,
                   num_devices=NCORES)
    io = {
        "q_t": nc.dram_tensor("q_t", [128, NG, KC, 512], BF16,
                              kind="ExternalInput"),
        "d_t": nc.dram_tensor("d_t", [128, NG, KC, 512], BF16,
                              kind="ExternalInput"),
        "q_n": nc.dram_tensor("q_n", [128, QT, H], BF16, kind="ExternalInput"),
        "d_n": nc.dram_tensor("d_n", [128, DT, H], BF16, kind="ExternalInput"),
        "qids": nc.dram_tensor("qids", [B, LQ], I32, kind="ExternalInput"),
        "qids_t": nc.dram_tensor("qids_t", [128, QT], I32,
                                 kind="ExternalInput"),
        "dmsk": nc.dram_tensor("dmsk", [128, 2 * DT], I32,
                               kind="ExternalInput"),
        "qcls": nc.dram_tensor("qcls", [B, H], BF16, kind="ExternalInput"),
        "qclsT": nc.dram_tensor("qclsT", [128, KC, B], BF16,
                                kind="ExternalInput"),
        "dcls": nc.dram_tensor("dcls", [L * MLOC, H], BF16,
                               kind="ExternalInput"),
        "dclsT": nc.dram_tensor("dclsT", [128, KC, L * MLOC], BF16,
                                kind="ExternalInput"),
        "logt": nc.dram_tensor("logt", [B, 1], F32, kind="ExternalInput"),
        "out": nc.dram_tensor("out", [B, MLOC], F32, kind="ExternalOutput"),
    }
    with tile.TileContext(nc) as tc:
        _emit(nc, tc, io)
    nc.compile()
    _CACHE["nc"] = nc
    return nc


BF16NP = ml_dtypes.bfloat16
FP8NP = ml_dtypes.float8_e4m3fn


def _to_groups(x2d):
    """(2048, 768) -> (128, 4, 6, 512) with [p, g, k, j] = x[g*512+j, k*128+p]."""
    return np.ascontiguousarray(
        x2d.reshape(NG, 512, KC, 128).transpose(3, 0, 2, 1))


def _to_ptiles(x2d):
    """(2048, 768) -> (128, 16, 768) with [p, c, h] = x[c*128+p, h]."""
    return np.ascontiguousarray(x2d.reshape(QT, 128, H).transpose(1, 0, 2))


def make_in_maps(q_tok, d_tok, q_cls, d_cls, log_inv_t, q_ids, d_ids,
                 d_punct_mask):
    qf = np.asarray(q_tok, np.float32).reshape(BQ, H)
    q_tb = _to_groups(qf.astype(BF16NP))
    q_n8 = _to_ptiles(qf.astype(BF16NP))
    qids = np.ascontiguousarray(np.asarray(q_ids, np.int32))
    qids_t = np.ascontiguousarray(qids.reshape(QT, 128).T)
    qcls = np.asarray(q_cls, np.float32)[-1].astype(BF16NP)
    qclsT = np.ascontiguousarray(qcls.reshape(B, KC, 128).transpose(2, 1, 0))
    logt = np.full((B, 1), np.float32(np.asarray(log_inv_t)), np.float32)
    d_tok = np.asarray(d_tok, np.float32)
    d_cls = np.asarray(d_cls, np.float32)
    d_ids = np.asarray(d_ids, np.int32)
    d_pun = np.asarray(d_punct_mask).astype(np.int32)
    in_maps = []
    for c in range(NCORES):
        sl = slice(c * MLOC, (c + 1) * MLOC)
        df = np.ascontiguousarray(d_tok[sl].reshape(DR, H))
        dcls_c = d_cls[:, sl, :].reshape(L * MLOC, H).astype(BF16NP)
        dmsk = np.concatenate(
            [d_ids[sl].reshape(DT, 128).T, d_pun[sl].reshape(DT, 128).T],
            axis=1)
        in_maps.append({
            "q_t": q_tb,
            "d_t": _to_groups(df.astype(BF16NP)),
            "q_n": q_n8,
            "d_n": _to_ptiles(df.astype(BF16NP)),
            "qids": qids,
            "qids_t": qids_t,
            "dmsk": np.ascontiguousarray(dmsk),
            "qcls": np.ascontiguousarray(qcls),
            "qclsT": qclsT,
            "dcls": np.ascontiguousarray(dcls_c),
            "dclsT": np.ascontiguousarray(
                dcls_c.reshape(L * MLOC, KC, 128).transpose(2, 1, 0)),
            "logt": logt,
        })
    return in_maps


_PERM = np.concatenate([np.arange(0, M, 2), np.arange(1, M, 2)])


def kernel(q_tok, d_tok, q_cls, d_cls, log_inv_t, q_ids, d_ids, d_punct_mask,
           **run_kwargs):
    nc = _build()
    in_maps = make_in_maps(q_tok, d_tok, q_cls, d_cls, log_inv_t, q_ids,
                           d_ids, d_punct_mask)
    res = bass_utils.run_bass_kernel_spmd(nc, in_maps,
                                          core_ids=list(range(NCORES)),
                                          **run_kwargs)
    full = np.concatenate([res.results[c]["out"] for c in range(NCORES)],
                          axis=1)
    out = full[:, _PERM]
    if run_kwargs:
        kernel.last_results = res
    return out


# revision 22
# speedup vs baseline: 1.0453x; 1.0453x over previous
"""Trainium2 Bass kernel for nn_CustomRetrieverModel (retrieval_knn).

Late-interaction retriever scoring:
  sim4d = l2n(q_tok) @ l2n(d_tok * punct).T  -> max over doc tokens
  -> valid-weighted mean over query tokens -> avg_sim (B, M)
  logits = shuffle(avg_sim) * shuffle(Wq) * exp(log_inv_t)
  with Wq from L2-normalized CLS vectors: (center - min cand)/2.

Sharding: data-parallel over the M (document) axis. Each of the 8 cores
scores all B=32 queries against M/8 = 8 docs; q-side inputs replicated,
host concatenates the per-core (B, 8) logits and applies the even/odd
column shuffle (a pure output permutation commutes with the elementwise
finale).

Device-side plan (per core), v4:
  - Matmul operands arrive HOST-PRE-TRANSPOSED in bf16 as
    (128, 4, 6, 512) = (h%128, col-group, h-chunk, col): the PE does no
    layout transposes and streams 1 row/cycle at 2.4 GHz.
  - The two *hardware* DGE rings (SP + Activation) run ~105-140 GB/s
    each and gate the start, so the first dT/qT group is split in half
    across both rings and every transfer is kick-ordered by its compute
    deadline; latency-tolerant bulk (last dT group, fp8 naturals for q
    norms, CLS, n_valid ids) rides the software DGE queues.
  - Row norms come from fp8-e4m3 natural-layout copies via ACT square +
    free-axis accumulate (norm error ~0.2%, well inside tolerance).
  - 1/||x|| uses the ACT Rsqrt table + one DVE Newton step.  Square,
    Copy and Rsqrt share one ACT table set (reciprocal_sqrt_and_small),
    so after an early warm-up there are no table reloads; the lone Exp
    runs first, while DMA is still streaming.
  - q is not normalized on device: max over doc tokens commutes with the
    positive row scale 1/||q||, folded into the weighted-sum weights.
  - d columns are scaled by mask/||d||: groups 1-3 in place on the DVE
    ahead of use; group 0 applies the scale to the PSUM sim block just
    before its reduce_max, so the first matmuls depend only on raw
    dT/qT arriving -- not on the normalization chain.
  - main matmul: per (q-tile, d-group) block, 6 bf16 matmuls of
    (128x128)@(128x512) accumulate in PSUM; DVE reduce_max per doc;
    weighted-sum matmul accumulates the (32, 8) sum_sim during the last
    group sweep.
  - CLS path in bf16: matmul first, separable row/column normalization
    applied to the (32, 24) result.
  - The Tile scheduler reorders per-engine streams using an optimistic
    DMA model, so everything with late-arriving inputs (q squares,
    build_W, CLS, finale) carries a tile_wait_until() floor to keep it
    from head-of-line blocking critical FIFO positions.
  - pad d tokens are zeroed (not -1e-9-masked): only changes the max
    when every real token sims below -1e-9, an O(1e-9) absolute effect.
"""

import sys

for _p in ("/opt/trn_rl_repo",):
    if _p not in sys.path:
        sys.path.append(_p)

import contextlib
import math

import numpy as np
import ml_dtypes

import concourse.bass as bass
import concourse.tile as tile
from concourse import bacc, mybir
import concourse.bass_utils as bass_utils

# ---- problem shape (hardcoded per spec) ----
B, LQ, M, LD, H, L = 32, 64, 64, 256, 768, 3
NCORES = 8
MLOC = M // NCORES          # 8 docs per core
BQ = B * LQ                 # 2048 query rows
DR = MLOC * LD              # 2048 doc-token rows per core
KC = H // 128               # 6 contraction chunks
QT = BQ // 128              # 16 q row tiles
DT = DR // 128              # 16 d row tiles
NG = 4                      # 512-wide column groups

EPS_DIV = 1e-10
LN2 = math.log(2.0)

F32 = mybir.dt.float32
BF16 = mybir.dt.bfloat16
FP8 = mybir.dt.float8e4
I32 = mybir.dt.int32


def _emit(nc, tc, io):
    q_t = io["q_t"].ap()          # (128, 4, 6, 512) bf16  q^T, replicated
    d_t = io["d_t"].ap()          # (128, 4, 6, 512) bf16  d^T shard
    q_n = io["q_n"].ap()          # (128, 16, 768) bf16 natural q (norms)
    d_n = io["d_n"].ap()          # (128, 16, 768) bf16 natural d (norms)
    qids = io["qids"].ap()        # (32, 64)   i32    replicated
    qcls = io["qcls"].ap()        # (32, 768)  bf16   q_cls[-1] natural
    qclsT = io["qclsT"].ap()      # (128, 6, 32) bf16 q_cls[-1]^T
    dcls = io["dcls"].ap()        # (24, 768)  bf16   d_cls shard natural
    dclsT = io["dclsT"].ap()      # (128, 6, 24) bf16 d_cls shard^T
    logt = io["logt"].ap()        # (32, 1)    f32    log_inv_t replicated
    out = io["out"].ap()          # (32, 8)    f32

    AF = mybir.ActivationFunctionType
    ALU = mybir.AluOpType

    ctx = contextlib.ExitStack()
    singles = ctx.enter_context(tc.tile_pool(name="singles", bufs=1))
    smalls = ctx.enter_context(tc.tile_pool(name="smalls", bufs=4))

    # ---------- constants (gpsimd only; no DMA) ----------
    identf = singles.tile([128, 128], F32)
    nc.gpsimd.memset(identf, 1.0)
    nc.gpsimd.affine_select(
        out=identf, in_=identf, pattern=[[-1, 128]], base=0,
        channel_multiplier=1, compare_op=ALU.is_equal, fill=0.0,
    )
    onesb = singles.tile([1, 128], BF16)
    nc.gpsimd.memset(onesb, 1.0)
    ones32 = singles.tile([1, 32], F32)
    nc.gpsimd.memset(ones32, 1.0)
    warm1 = singles.tile([1, 1], F32)
    nc.gpsimd.memset(warm1, 1.0)
    bln2 = singles.tile([32, 1], F32)
    nc.gpsimd.memset(bln2, -LN2)
    W = singles.tile([128, QT, 2 * QT], F32)  # block-diagonal (128, 16, 32)
    nc.gpsimd.memset(W, 0.0)

    # ---------- persistent SBUF ----------
    qss = singles.tile([128, QT], F32)
    dss = singles.tile([128, DT], F32)
    qT = singles.tile([128, NG, KC, 512], BF16)
    dT = singles.tile([128, NG, KC, 512], BF16)
    S = singles.tile([128, NG, 512], BF16)
    dmsk = singles.tile([128, 2 * DT], I32)   # [dids | dpunct] packed
    qid_t = singles.tile([128, QT], I32)
    qid_n = smalls.tile([32, 64], I32)
    lt_t = smalls.tile([32, 1], F32)
    dn0 = singles.tile([128, 4, H], BF16)      # natural d, group 0
    dn123 = singles.tile([128, 12, H], BF16)   # natural d, groups 1-3
    qn = singles.tile([128, QT, H], BF16)      # natural q, all tiles

    # -------- rsqrt: ACT Sqrt (table shared with Square) + DVE polish ----
    def rsqrt_nt(dst, ss, pool):
        shape = list(ss.shape)
        n0 = pool.tile(shape, F32, tag="rsq_n0")
        nc.scalar.sqrt(n0, ss)
        nc.vector.tensor_scalar_max(n0, n0, 1e-30)
        r0 = pool.tile(shape, F32, tag="rsq_r0")
        nc.vector.reciprocal(r0, n0)
        t = pool.tile(shape, F32, tag="rsq_t")
        nc.vector.tensor_mul(t, ss, r0)
        nc.vector.tensor_add(t, t, n0)
        nc.vector.tensor_scalar(t, t, 0.5, 1e-12, op0=ALU.mult, op1=ALU.max)
        nc.vector.reciprocal(dst, t)

    with tc.tile_pool(name="sqscr", bufs=2) as sqscr, \
         tc.tile_pool(name="clsp", bufs=1) as clsp, \
         tc.tile_pool(name="mm", bufs=5, space="PSUM") as mm_pool, \
         tc.tile_pool(name="aux", bufs=2, space="PSUM") as aux_ps, \
         tc.tile_pool(name="ws", bufs=1, space="PSUM") as ws_pool, \
         tc.tile_pool(name="maxs", bufs=QT) as maxs_pool:

        maxs_tiles = [None] * QT

        def d_squares(src, cols, g):
            for j in range(4):
                c = 4 * g + j
                scr = sqscr.tile([128, H], BF16, tag="sq")
                nc.scalar.activation(scr, src[:, cols + j, :], AF.Square,
                                     accum_out=dss[:, c:c + 1])

        def q_squares(gg):
            for j in range(4):
                c = 4 * gg + j
                scr = sqscr.tile([128, H], BF16, tag="sq")
                nc.scalar.activation(scr, qn[:, c, :], AF.Square,
                                     accum_out=qss[:, c:c + 1])

        def d_scale(g):
            # s = mask / ||d|| for this group's 4 tiles
            sl = slice(4 * g, 4 * g + 4)
            s4 = smalls.tile([128, 4], F32, tag="s4")
            rsqrt_nt(s4, dss[:, sl], smalls)
            nc.vector.tensor_mul(s4, s4, dmask_f[:, sl])
            # relayout (128, 4) -> scale row (1, 512) -> broadcast (128, 512)
            st = smalls.tile([1, 512], BF16, tag="st")
            for c in range(4):
                stp = aux_ps.tile([1, 128], F32, tag="aux")
                nc.tensor.transpose(stp, s4[:, c:c + 1], identf)
                nc.vector.tensor_copy(st[0:1, c * 128:(c + 1) * 128], stp)
            bc = aux_ps.tile([128, 512], F32, tag="aux")
            nc.tensor.matmul(bc, onesb, st, start=True, stop=True)
            nc.scalar.copy(S[:, g], bc)
            if g > 0:
                # in-place column scale of this group's dT chunks
                for k in range(KC):
                    nc.vector.tensor_mul(dT[:, g, k], dT[:, g, k], S[:, g])

        def main_block(qc, g):
            if maxs_tiles[qc] is None:
                maxs_tiles[qc] = maxs_pool.tile([128, MLOC], F32, tag="maxs",
                                                name=f"maxs{qc}")
            qg, sub = divmod(qc, 4)
            ps = mm_pool.tile([128, 512], F32, tag="mm")
            for k in range(KC):
                nc.tensor.matmul(ps, qT[:, qg, k, sub * 128:(sub + 1) * 128],
                                 dT[:, g, k],
                                 start=(k == 0), stop=(k == KC - 1))
            if g == 0:
                # group 0 runs on raw dT; apply mask/||d|| on the sim block
                nc.vector.tensor_mul(ps, ps, S[:, 0])
            nc.vector.reduce_max(
                maxs_tiles[qc][:, 2 * g:2 * g + 2],
                ps[:].rearrange("p (d l) -> p d l", l=LD),
                axis=mybir.AxisListType.X)

        def build_W():
            # q_valid mask and weighted-sum weights q_valid/||q||
            qv = smalls.tile([128, QT], F32, name="qv")
            nc.vector.tensor_scalar(qv, qid_t, 0.0, None, op0=ALU.is_equal)
            nc.vector.tensor_scalar(qv, qv, -1.0, 1.0,
                                    op0=ALU.mult, op1=ALU.add)
            rq = smalls.tile([128, QT], F32, name="rq")
            rsqrt_nt(rq, qss, smalls)
            wqw = smalls.tile([128, QT], F32, name="wqw")
            nc.vector.tensor_mul(wqw, qv, rq)
            for c in range(QT):
                nc.gpsimd.tensor_copy(W[0:64, c, 2 * c:2 * c + 1],
                                      wqw[0:64, c:c + 1])
                nc.gpsimd.tensor_copy(W[64:128, c, 2 * c + 1:2 * c + 2],
                                      wqw[64:128, c:c + 1])

        def cls_block():
            qcn = clsp.tile([32, H], BF16, tag="qcn")
            nc.scalar.dma_start(qcn, qcls)
            qcT = clsp.tile([128, KC, 32], BF16, tag="qcT")
            nc.scalar.dma_start(qcT, qclsT)
            dcn = clsp.tile([24, H], BF16, tag="dcn")
            nc.sync.dma_start(dcn, dcls)
            dcT = clsp.tile([128, KC, 24], BF16, tag="dcT")
            nc.sync.dma_start(dcT, dclsT)

            qcss = smalls.tile([32, 1], F32, tag="qcss")
            scr1 = clsp.tile([32, H], BF16, tag="clsscr")
            nc.scalar.activation(scr1, qcn, AF.Square, accum_out=qcss)
            dcss = smalls.tile([24, 1], F32, tag="dcss")
            scr2 = clsp.tile([24, H], BF16, tag="clsscr24")
            nc.scalar.activation(scr2, dcn, AF.Square, accum_out=dcss)
            rqc = smalls.tile([32, 1], F32, tag="rqc")
            rsqrt_nt(rqc, qcss, smalls)
            rdc = smalls.tile([24, 1], F32, tag="rdc")
            rsqrt_nt(rdc, dcss, smalls)

            # raw (32, 24) = qcT.T @ dcT, normalized afterwards (separable)
            cp = aux_ps.tile([32, 24], F32, tag="aux")
            for k in range(KC):
                nc.tensor.matmul(cp, qcT[:, k], dcT[:, k],
                                 start=(k == 0), stop=(k == KC - 1))
            raw = smalls.tile([32, 24], F32, tag="raw")
            nc.scalar.copy(raw, cp)
            nc.vector.tensor_scalar(raw, raw, rqc, None, op0=ALU.mult)
            # rdc (24,1) -> row (1,24) -> broadcast (32,24)
            rtp = aux_ps.tile([1, 24], F32, tag="aux")
            nc.tensor.transpose(rtp, rdc, identf[0:24, 0:24])
            rdT = smalls.tile([1, 24], F32, tag="rdT")
            nc.vector.tensor_copy(rdT, rtp)
            bcp = aux_ps.tile([32, 24], F32, tag="aux")
            nc.tensor.matmul(bcp, ones32, rdT, start=True, stop=True)
            rdB = smalls.tile([32, 24], F32, tag="rdB")
            nc.scalar.copy(rdB, bcp)
            nc.vector.tensor_mul(raw, raw, rdB)

            mind = smalls.tile([32, 8], F32, tag="mind")
            nc.vector.tensor_tensor(mind, raw[:, 0:8], raw[:, 8:16],
                                    op=ALU.min)
            wq2 = smalls.tile([32, 8], F32, tag="wq2")  # center - min_doc
            nc.vector.tensor_sub(wq2, raw[:, 16:24], mind)
            return wq2

        ws_ps = ws_pool.tile([32, MLOC], F32)   # sum_sim accumulator

        # ---------- DMA kicks, deadline-ordered across both HW rings ----
        def w(ms):
            return tc.tile_wait_until(ms)

        # software DGE queue carries the bulk (it fans out over the most
        # DMA engines, ~224 GB/s observed); deadline-ordered FIFO.  High
        # priority puts the kicks ahead of the gpsimd memsets.
        with tc.high_priority():
            nc.gpsimd.dma_start(dT[:, 0], d_t[:, 0])
            with w(0.0008):
                nc.gpsimd.dma_start(qT[:, 1], q_t[:, 1])
            with w(0.0016):
                nc.gpsimd.dma_start(qT[:, 2], q_t[:, 2])
            with w(0.0030):
                nc.gpsimd.dma_start(qT[:, 3], q_t[:, 3])
            with w(0.0080):
                nc.gpsimd.dma_start(dT[:, 1], d_t[:, 1])
            with w(0.0140):
                nc.gpsimd.dma_start(dT[:, 2], d_t[:, 2])
            with w(0.0180):
                nc.gpsimd.dma_start(dT[:, 3], d_t[:, 3])
        # SP HW ring: small/early-critical + d naturals
        nc.sync.dma_start(lt_t, logt)
        nc.sync.dma_start(dmsk, io["dmsk"].ap())
        nc.sync.dma_start(qT[:, 0, 0:3], q_t[:, 0, 0:3])
        with w(0.0010):
            nc.sync.dma_start(dn0, d_n[:, 0:4, :])
        with w(0.0050):
            nc.sync.dma_start(dn123, d_n[:, 4:16, :])
        with w(0.0500):
            nc.sync.dma_start(qid_n, qids)
        # Activation HW ring: second qT0 half + q naturals
        nc.scalar.dma_start(qT[:, 0, 3:6], q_t[:, 0, 3:6])
        nc.scalar.dma_start(qid_t, io["qids_t"].ap())
        with w(0.0120):
            nc.scalar.dma_start(qn, q_n)

        # exp + Rsqrt table warm-up while DMA streams (one switch, early)
        it_half = smalls.tile([32, 1], F32, tag="ith")
        nc.scalar.activation(it_half, lt_t, AF.Exp, bias=bln2, scale=1.0)
        warm2 = smalls.tile([1, 1], F32, tag="warm2")
        nc.scalar.sqrt(warm2, warm1)

        # dmask = (d_ids != 0) * punct  -- first DVE work, data lands early
        dmask_f = singles.tile([128, DT], F32)
        nc.vector.tensor_scalar(dmask_f, dmsk[:, 0:DT], 0.0, None,
                                op0=ALU.is_equal)
        nc.vector.tensor_scalar(dmask_f, dmask_f, -1.0, 1.0,
                                op0=ALU.mult, op1=ALU.add)
        pun_f = smalls.tile([128, DT], F32, tag="punf")
        nc.vector.tensor_copy(pun_f, dmsk[:, DT:2 * DT])
        nc.vector.tensor_mul(dmask_f, dmask_f, pun_f)

        # group-0 normalization chain (overlaps the first main matmuls)
        d_squares(dn0, 0, 0)
        with w(0.004):
            d_scale(0)

        wq2 = None
        for g in range(NG):
            for qc in range(QT):
                main_block(qc, g)
                if g == 3:
                    nc.tensor.matmul(ws_ps, W[:, qc, :], maxs_tiles[qc],
                                     start=(qc == 0), stop=(qc == QT - 1))
                if g == 0:
                    if qc == 1:
                        d_squares(dn123, 0, 1)
                    elif qc == 5:
                        d_scale(1)
                    elif qc == 7:
                        with w(0.020):
                            q_squares(0)
                    elif qc == 9:
                        with w(0.024):
                            q_squares(1)
                    elif qc == 11:
                        with w(0.028):
                            q_squares(2)
                    elif qc == 13:
                        with w(0.032):
                            q_squares(3)
                elif g == 1:
                    if qc == 1:
                        d_squares(dn123, 4, 2)
                    elif qc == 3:
                        d_scale(2)
                    elif qc == 10:
                        with w(0.045):
                            wq2 = cls_block()
                elif g == 2:
                    if qc == 1:
                        d_squares(dn123, 8, 3)
                    elif qc == 3:
                        d_scale(3)
            if g == 0:
                with w(0.038):
                    build_W()

        # ---------- finale ----------
        with w(0.080):
            # n_valid: 64 - sum(q_ids == 0)
            qv_n = smalls.tile([32, 64], F32, tag="qvn")
            nc.vector.tensor_scalar(qv_n, qid_n, 0.0, None, op0=ALU.is_equal)
            nv_eq = smalls.tile([32, 1], F32, tag="nveq")
            nc.vector.reduce_sum(nv_eq, qv_n, axis=mybir.AxisListType.X)
            rnv = smalls.tile([32, 1], F32, tag="rnv")
            nc.vector.tensor_scalar(rnv, nv_eq, -1.0, 64.0 + EPS_DIV,
                                    op0=ALU.mult, op1=ALU.add)
            nc.vector.reciprocal(rnv, rnv)

        avg = smalls.tile([32, 8], F32, tag="avg")
        nc.vector.tensor_scalar(avg, ws_ps, rnv, None, op0=ALU.mult)
        nc.vector.tensor_mul(avg, avg, wq2)
        outt = smalls.tile([32, 8], F32, tag="outt")
        nc.vector.tensor_scalar(outt, avg, it_half, None, op0=ALU.mult)
        nc.sync.dma_start(out, outt)

    ctx.close()


_CACHE = {}


def _build():
    if "nc" in _CACHE:
        return _CACHE["nc"]
    nc = bacc.Bacc("TRN2", target_bir_lowering=False, debug=False,
                   num_devices=NCORES)
    io = {
        "q_t": nc.dram_tensor("q_t", [128, NG, KC, 512], BF16,
                              kind="ExternalInput"),
        "d_t": nc.dram_tensor("d_t", [128, NG, KC, 512], BF16,
                              kind="ExternalInput"),
        "q_n": nc.dram_tensor("q_n", [128, QT, H], BF16, kind="ExternalInput"),
        "d_n": nc.dram_tensor("d_n", [128, DT, H], BF16, kind="ExternalInput"),
        "qids": nc.dram_tensor("qids", [B, LQ], I32, kind="ExternalInput"),
        "qids_t": nc.dram_tensor("qids_t", [128, QT], I32,
                                 kind="ExternalInput"),
        "dmsk": nc.dram_tensor("dmsk", [128, 2 * DT], I32,
                               kind="ExternalInput"),
        "qcls": nc.dram_tensor("qcls", [B, H], BF16, kind="ExternalInput"),
        "qclsT": nc.dram_tensor("qclsT", [128, KC, B], BF16,
                                kind="ExternalInput"),
        "dcls": nc.dram_tensor("dcls", [L * MLOC, H], BF16,
                               kind="ExternalInput"),
        "dclsT": nc.dram_tensor("dclsT", [128, KC, L * MLOC], BF16,
                                kind="ExternalInput"),
        "logt": nc.dram_tensor("logt", [B, 1], F32, kind="ExternalInput"),
        "out": nc.dram_tensor("out", [B, MLOC], F32, kind="ExternalOutput"),
    }
    with tile.TileContext(nc) as tc:
        _emit(nc, tc, io)
    nc.compile()
    _CACHE["nc"] = nc
    return nc


BF16NP = ml_dtypes.bfloat16
FP8NP = ml_dtypes.float8_e4m3fn


def _to_groups(x2d):
    """(2048, 768) -> (128, 4, 6, 512) with [p, g, k, j] = x[g*512+j, k*128+p]."""
    return np.ascontiguousarray(
        x2d.reshape(NG, 512, KC, 128).transpose(3, 0, 2, 1))


def _to_ptiles(x2d):
    """(2048, 768) -> (128, 16, 768) with [p, c, h] = x[c*128+p, h]."""
    return np.ascontiguousarray(x2d.reshape(QT, 128, H).transpose(1, 0, 2))


def make_in_maps(q_tok, d_tok, q_cls, d_cls, log_inv_t, q_ids, d_ids,
                 d_punct_mask):
    qf = np.asarray(q_tok, np.float32).reshape(BQ, H)
    q_tb = _to_groups(qf.astype(BF16NP))
    q_n8 = _to_ptiles(qf.astype(BF16NP))
    qids = np.ascontiguousarray(np.asarray(q_ids, np.int32))
    qids_t = np.ascontiguousarray(qids.reshape(QT, 128).T)
    qcls = np.asarray(q_cls, np.float32)[-1].astype(BF16NP)
    qclsT = np.ascontiguousarray(qcls.reshape(B, KC, 128).transpose(2, 1, 0))
    logt = np.full((B, 1), np.float32(np.asarray(log_inv_t)), np.float32)
    d_tok = np.asarray(d_tok, np.float32)
    d_cls = np.asarray(d_cls, np.float32)
    d_ids = np.asarray(d_ids, np.int32)
    d_pun = np.asarray(d_punct_mask).astype(np.int32)
    in_maps = []
    for c in range(NCORES):
        sl = slice(c * MLOC, (c + 1) * MLOC)
        df = np.ascontiguousarray(d_tok[sl].reshape(DR, H))
        dcls_c = d_cls[:, sl, :].reshape(L * MLOC, H).astype(BF16NP)
        dmsk = np.concatenate(
            [d_ids[sl].reshape(DT, 128).T, d_pun[sl].reshape(DT, 128).T],
            axis=1)
        in_maps.append({
            "q_t": q_tb,
            "d_t": _to_groups(df.astype(BF16NP)),
            "q_n": q_n8,
            "d_n": _to_ptiles(df.astype(BF16NP)),
            "qids": qids,
            "qids_t": qids_t,
            "dmsk": np.ascontiguousarray(dmsk),
            "qcls": np.ascontiguousarray(qcls),
            "qclsT": qclsT,
            "dcls": np.ascontiguousarray(dcls_c),
            "dclsT": np.ascontiguousarray(
                dcls_c.reshape(L * MLOC, KC, 128).transpose(2, 1, 0)),
            "logt": logt,
        })
    return in_maps


_PERM = np.concatenate([np.arange(0, M, 2), np.arange(1, M, 2)])


def kernel(q_tok, d_tok, q_cls, d_cls, log_inv_t, q_ids, d_ids, d_punct_mask,
           **run_kwargs):
    nc = _build()
    in_maps = make_in_maps(q_tok, d_tok, q_cls, d_cls, log_inv_t, q_ids,
                           d_ids, d_punct_mask)
    res = bass_utils.run_bass_kernel_spmd(nc, in_maps,
                                          core_ids=list(range(NCORES)),
                                          **run_kwargs)
    full = np.concatenate([res.results[c]["out"] for c in range(NCORES)],
                          axis=1)
    out = full[:, _PERM]
    if run_kwargs:
        kernel.last_results = res
    return out
